# revision 13
# baseline (speedup 1.0000x reference)
"""Green's function layer kernel for Trainium2 (8 NeuronCores, data-parallel over batch).

Math: reference computes, per batch b,
    G_b = inv((w_b + i*eta) I - H_sym),  output |G_b|,
with H_sym = 0.5(H+H^T) shared across the batch and w_b a scalar from a tiny MLP.

Since H_sym is real symmetric and shared, eigendecompose once on host:
    H_sym = Q diag(lam) Q^T  =>  G_b = Q diag(1/(w_b - lam + i*eta)) Q^T.
With c_b = 1/(w_b - lam + i*eta) (complex vector), the per-batch work becomes two
real [1024x1024] matmuls (real and imaginary parts) plus an elementwise abs:
    Re(G_b) = Q diag(c_re) Q^T,  Im(G_b) = Q diag(c_im) Q^T,
    |G_b| = sqrt(Re^2 + Im^2).
Each core handles 4 of the 32 batches; Q^T is replicated.
"""

import numpy as np

ETA = 0.01
B, NG, HID = 32, 1024, 64
NCORES = 8
BPC = B // NCORES  # batches per core
P = 128
KT = NG // P   # 8 contraction tiles
MT = NG // P   # 8 output row tiles
NW = 512       # matmul moving free dim (one fp32 PSUM bank)
NJ = NG // NW  # 2 output col tiles

USE_F32R = True
N2 = 256        # half-tile: psum holds [re(256) | im(256)]
NJ4 = NG // N2  # 4 col tiles of 256

# Output is symmetric: keep tile (mi, nj) iff mi < 2*nj + 2 (covers the
# upper triangle); the rest is mirrored on the host.
KEEP = [(mi, nj) for mi in range(MT) for nj in range(NJ4) if mi < 2 * nj + 2]
MISS = [(mi, nj) for mi in range(MT) for nj in range(NJ4) if mi >= 2 * nj + 2]

_CACHE = {}


def _build_nc():
    from concourse import bacc
    import concourse.mybir as mybir
    import concourse.tile as tile

    f32 = mybir.dt.float32
    f32r = mybir.dt.float32r

    nc = bacc.Bacc("TRN2", target_bir_lowering=False, debug=False, num_devices=NCORES)

    qt_d = nc.dram_tensor("qt", [NG, NG], f32, kind="ExternalInput").ap()
    cre_d = nc.dram_tensor("cre", [BPC, NG], f32, kind="ExternalInput").ap()
    cim_d = nc.dram_tensor("cim", [BPC, NG], f32, kind="ExternalInput").ap()
    out_d = nc.dram_tensor("out", [BPC, NG, NG], f32, kind="ExternalOutput").ap()

    # DRAM views: k index on partitions.
    qt_v = qt_d.rearrange("(t p) m -> p t m", p=P)       # [128, KT, NG]
    cre_v = cre_d.rearrange("b (t p) -> b p t", p=P)     # [BPC, 128, KT]
    cim_v = cim_d.rearrange("b (t p) -> b p t", p=P)

    mdt = f32r if USE_F32R else f32

    with tile.TileContext(nc) as tc:
        with (
            tc.tile_pool(name="qtp", bufs=1) as qtp,
            tc.tile_pool(name="stg", bufs=2) as stg,
            tc.tile_pool(name="scp", bufs=2) as scp,
            tc.tile_pool(name="cvp", bufs=2) as cvp,
            tc.tile_pool(name="otp", bufs=3) as otp,
            tc.tile_pool(name="psp", bufs=6, space="PSUM") as psp,
        ):
            # c vectors first (tiny, gate the first scale ops)
            cts = []
            for b in range(BPC):
                cre_t = cvp.tile([P, KT], f32, tag=f"cre{b}")
                cim_t = cvp.tile([P, KT], f32, tag=f"cim{b}")
                nc.sync.dma_start(cre_t[:], cre_v[b])
                nc.sync.dma_start(cim_t[:], cim_v[b])
                cts.append((cre_t, cim_t))

            # qt: split each k-tile into 8 column chunks so chunks spread
            # across DMA queues and early k-tiles land fast
            qt = qtp.tile([P, KT, NG], mdt)
            CH = NG // 8
            for ki in range(KT):
                for c in range(8):
                    cs = slice(c * CH, (c + 1) * CH)
                    nc.sync.dma_start(qt[:, ki, cs], qt_v[:, ki, cs].bitcast(mdt))

            for b in range(BPC):
                cre_t, cim_t = cts[b]

                # scat[:, ki, nj, 0:256] = cre * qt cols, [..., 256:512] = cim * qt
                scat = scp.tile([P, KT, NJ4, 2 * N2], mdt, tag="scat")
                for ki in range(KT):
                    qv = qt[:, ki, :].rearrange("p (a b) -> p a b", b=N2)
                    nc.vector.tensor_scalar_mul(
                        scat[:, ki, :, 0:N2], qv, cre_t[:, ki : ki + 1]
                    )
                    nc.vector.tensor_scalar_mul(
                        scat[:, ki, :, N2 : 2 * N2], qv, cim_t[:, ki : ki + 1]
                    )

                for mi, nj in KEEP:
                    ms = slice(mi * P, (mi + 1) * P)
                    ps = psp.tile([P, 2 * N2], f32, tag="ps")
                    for ki in range(KT):
                        nc.tensor.matmul(
                            ps[:],
                            qt[:, ki, ms],
                            scat[:, ki, nj, :],
                            start=(ki == 0),
                            stop=(ki == KT - 1),
                        )
                    sq = otp.tile([P, 2 * N2], f32, tag="sq")
                    nc.scalar.square(sq[:], ps[:])
                    t = otp.tile([P, N2], f32, tag="t")
                    nc.vector.tensor_add(t[:], sq[:, 0:N2], sq[:, N2 : 2 * N2])
                    o = otp.tile([P, N2], f32, tag="o")
                    nc.scalar.sqrt(o[:], t[:])
                    h = N2 // 2
                    c0 = nj * N2
                    nc.sync.dma_start(out_d[b, ms, c0 : c0 + h], o[:, 0:h])
                    nc.sync.dma_start(out_d[b, ms, c0 + h : c0 + N2], o[:, h:N2])

    nc.compile()
    return nc


def _host_prep(gene_state, H, W1, b1, W2, b2):
    # omega_net MLP -> per-batch scalar w (fp32, matching the jax reference)
    gs = gene_state.astype(np.float32).reshape(-1, HID)
    h = gs @ W1.astype(np.float32) + b1.astype(np.float32)
    h = h * (1.0 / (1.0 + np.exp(-h, dtype=np.float32)))  # SiLU
    omega = (h @ W2.astype(np.float32) + b2.astype(np.float32)).reshape(B, NG)
    w = omega.mean(axis=1)  # [B]

    Hs = 0.5 * (H.astype(np.float64) + H.astype(np.float64).T)
    lam, Q = np.linalg.eigh(Hs)  # Hs = Q diag(lam) Q^T

    d = w.astype(np.float64)[:, None] - lam[None, :]  # [B, NG]
    den = d * d + ETA * ETA
    cre = (d / den).astype(np.float32)
    cim = (-ETA / den).astype(np.float32)
    qt = np.ascontiguousarray(Q.T.astype(np.float32))  # [k, n]
    return qt, cre, cim


def kernel(gene_state, H, W1, b1, W2, b2):
    from concourse.bass_utils import run_bass_kernel_spmd

    qt, cre, cim = _host_prep(gene_state, H, W1, b1, W2, b2)

    if "nc" not in _CACHE:
        _CACHE["nc"] = _build_nc()
    nc = _CACHE["nc"]

    in_maps = [
        {
            "qt": qt,
            "cre": np.ascontiguousarray(cre[c * BPC : (c + 1) * BPC]),
            "cim": np.ascontiguousarray(cim[c * BPC : (c + 1) * BPC]),
        }
        for c in range(NCORES)
    ]
    res = run_bass_kernel_spmd(nc, in_maps, core_ids=list(range(NCORES)))
    out = np.concatenate([r["out"] for r in res.results], axis=0)
    # Mirror the skipped lower-triangle tiles from the computed upper ones.
    for mi, nj in MISS:
        r0, r1 = mi * P, (mi + 1) * P
        c0, c1 = nj * N2, (nj + 1) * N2
        out[:, r0:r1, c0:c1] = out[:, c0:c1, r0:r1].swapaxes(1, 2)
    return out


# revision 15
# speedup vs baseline: 1.1286x; 1.1286x over previous
"""Green's function layer kernel for Trainium2 (8 NeuronCores, data-parallel over batch).

Math: reference computes, per batch b,
    G_b = inv((w_b + i*eta) I - H_sym),  output |G_b|,
with H_sym = 0.5(H+H^T) shared across the batch and w_b a scalar from a tiny MLP.

Since H_sym is real symmetric and shared, eigendecompose once on host:
    H_sym = Q diag(lam) Q^T  =>  G_b = Q diag(1/(w_b - lam + i*eta)) Q^T.
With c_b = 1/(w_b - lam + i*eta) (complex vector), the per-batch work becomes two
real [1024x1024] matmuls (real and imaginary parts) plus an elementwise abs:
    Re(G_b) = Q diag(c_re) Q^T,  Im(G_b) = Q diag(c_im) Q^T,
    |G_b| = sqrt(Re^2 + Im^2).
Each core handles 4 of the 32 batches; Q^T is replicated.
"""

import numpy as np

ETA = 0.01
B, NG, HID = 32, 1024, 64
NCORES = 8
BPC = B // NCORES  # batches per core
P = 128
KT = NG // P   # 8 contraction tiles
MT = NG // P   # 8 output row tiles
NW = 512       # matmul moving free dim (one fp32 PSUM bank)
NJ = NG // NW  # 2 output col tiles

USE_F32R = True
N2 = 256        # half-tile: psum holds [re(256) | im(256)]
NJ4 = NG // N2  # 4 col tiles of 256

# Output is symmetric: keep tile (mi, nj) iff mi < 2*nj + 2 (covers the
# upper triangle); the rest is mirrored on the host.
KEEP = [(mi, nj) for mi in range(MT) for nj in range(NJ4) if mi < 2 * nj + 2]
MISS = [(mi, nj) for mi in range(MT) for nj in range(NJ4) if mi >= 2 * nj + 2]

_CACHE = {}


def _build_nc():
    from concourse import bacc
    import concourse.mybir as mybir
    import concourse.tile as tile

    f32 = mybir.dt.float32
    f32r = mybir.dt.float32r

    nc = bacc.Bacc("TRN2", target_bir_lowering=False, debug=False, num_devices=NCORES)

    qt_d = nc.dram_tensor("qt", [NG, NG], f32, kind="ExternalInput").ap()
    cre_d = nc.dram_tensor("cre", [BPC, NG], f32, kind="ExternalInput").ap()
    cim_d = nc.dram_tensor("cim", [BPC, NG], f32, kind="ExternalInput").ap()
    out_d = nc.dram_tensor("out", [BPC, NG, NG], f32, kind="ExternalOutput").ap()

    # DRAM views: k index on partitions.
    qt_v = qt_d.rearrange("(t p) m -> p t m", p=P)       # [128, KT, NG]
    cre_v = cre_d.rearrange("b (t p) -> b p t", p=P)     # [BPC, 128, KT]
    cim_v = cim_d.rearrange("b (t p) -> b p t", p=P)

    mdt = f32r if USE_F32R else f32

    with tile.TileContext(nc) as tc:
        with (
            tc.tile_pool(name="qtp", bufs=1) as qtp,
            tc.tile_pool(name="stg", bufs=2) as stg,
            tc.tile_pool(name="scp", bufs=2) as scp,
            tc.tile_pool(name="cvp", bufs=2) as cvp,
            tc.tile_pool(name="otp", bufs=3) as otp,
            tc.tile_pool(name="psp", bufs=6, space="PSUM") as psp,
        ):
            # c vectors first (tiny, gate the first scale ops)
            cts = []
            for b in range(BPC):
                cre_t = cvp.tile([P, KT], f32, tag=f"cre{b}")
                cim_t = cvp.tile([P, KT], f32, tag=f"cim{b}")
                nc.sync.dma_start(cre_t[:], cre_v[b])
                nc.sync.dma_start(cim_t[:], cim_v[b])
                cts.append((cre_t, cim_t))

            # qt: split each k-tile into 8 column chunks so chunks spread
            # across DMA queues and early k-tiles land fast
            qt = qtp.tile([P, KT, NG], mdt)
            CH = NG // 4  # 256-col chunks keep 1KB DMA packets
            for ki in range(KT):
                for c in range(4):
                    cs = slice(c * CH, (c + 1) * CH)
                    nc.sync.dma_start(qt[:, ki, cs], qt_v[:, ki, cs].bitcast(mdt))

            for b in range(BPC):
                cre_t, cim_t = cts[b]

                # scat[:, ki, nj, 0:256] = cre * qt cols, [..., 256:512] = cim * qt
                scat = scp.tile([P, KT, NJ4, 2 * N2], mdt, tag="scat")
                for ki in range(KT):
                    qv = qt[:, ki, :].rearrange("p (a b) -> p a b", b=N2)
                    nc.vector.tensor_scalar_mul(
                        scat[:, ki, :, 0:N2], qv, cre_t[:, ki : ki + 1]
                    )
                    nc.vector.tensor_scalar_mul(
                        scat[:, ki, :, N2 : 2 * N2], qv, cim_t[:, ki : ki + 1]
                    )

                for mi, nj in KEEP:
                    ms = slice(mi * P, (mi + 1) * P)
                    ps = psp.tile([P, 2 * N2], f32, tag="ps")
                    for ki in range(KT):
                        nc.tensor.matmul(
                            ps[:],
                            qt[:, ki, ms],
                            scat[:, ki, nj, :],
                            start=(ki == 0),
                            stop=(ki == KT - 1),
                        )
                    sq = otp.tile([P, 2 * N2], f32, tag="sq")
                    nc.scalar.square(sq[:], ps[:])
                    t = otp.tile([P, N2], f32, tag="t")
                    nc.vector.tensor_add(t[:], sq[:, 0:N2], sq[:, N2 : 2 * N2])
                    o = otp.tile([P, N2], f32, tag="o")
                    nc.scalar.sqrt(o[:], t[:])
                    nc.sync.dma_start(out_d[b, ms, nj * N2 : (nj + 1) * N2], o[:])

    nc.compile()
    return nc


def _host_prep(gene_state, H, W1, b1, W2, b2):
    # omega_net MLP -> per-batch scalar w (fp32, matching the jax reference)
    gs = gene_state.astype(np.float32).reshape(-1, HID)
    h = gs @ W1.astype(np.float32) + b1.astype(np.float32)
    h = h * (1.0 / (1.0 + np.exp(-h, dtype=np.float32)))  # SiLU
    omega = (h @ W2.astype(np.float32) + b2.astype(np.float32)).reshape(B, NG)
    w = omega.mean(axis=1)  # [B]

    Hs = 0.5 * (H.astype(np.float64) + H.astype(np.float64).T)
    lam, Q = np.linalg.eigh(Hs)  # Hs = Q diag(lam) Q^T

    d = w.astype(np.float64)[:, None] - lam[None, :]  # [B, NG]
    den = d * d + ETA * ETA
    cre = (d / den).astype(np.float32)
    cim = (-ETA / den).astype(np.float32)
    qt = np.ascontiguousarray(Q.T.astype(np.float32))  # [k, n]
    return qt, cre, cim


def kernel(gene_state, H, W1, b1, W2, b2):
    from concourse.bass_utils import run_bass_kernel_spmd

    qt, cre, cim = _host_prep(gene_state, H, W1, b1, W2, b2)

    if "nc" not in _CACHE:
        _CACHE["nc"] = _build_nc()
    nc = _CACHE["nc"]

    in_maps = [
        {
            "qt": qt,
            "cre": np.ascontiguousarray(cre[c * BPC : (c + 1) * BPC]),
            "cim": np.ascontiguousarray(cim[c * BPC : (c + 1) * BPC]),
        }
        for c in range(NCORES)
    ]
    res = run_bass_kernel_spmd(nc, in_maps, core_ids=list(range(NCORES)))
    out = np.concatenate([r["out"] for r in res.results], axis=0)
    # Mirror the skipped lower-triangle tiles from the computed upper ones.
    for mi, nj in MISS:
        r0, r1 = mi * P, (mi + 1) * P
        c0, c1 = nj * N2, (nj + 1) * N2
        out[:, r0:r1, c0:c1] = out[:, c0:c1, r0:r1].swapaxes(1, 2)
    return out


# revision 17
# speedup vs baseline: 1.1410x; 1.0110x over previous
"""Green's function layer kernel for Trainium2 (8 NeuronCores, data-parallel over batch).

Math: reference computes, per batch b,
    G_b = inv((w_b + i*eta) I - H_sym),  output |G_b|,
with H_sym = 0.5(H+H^T) shared across the batch and w_b a scalar from a tiny MLP.

Since H_sym is real symmetric and shared, eigendecompose once on host:
    H_sym = Q diag(lam) Q^T  =>  G_b = Q diag(1/(w_b - lam + i*eta)) Q^T.
With c_b = 1/(w_b - lam + i*eta) (complex vector), the per-batch work becomes two
real [1024x1024] matmuls (real and imaginary parts) plus an elementwise abs:
    Re(G_b) = Q diag(c_re) Q^T,  Im(G_b) = Q diag(c_im) Q^T,
    |G_b| = sqrt(Re^2 + Im^2).
Each core handles 4 of the 32 batches; Q^T is replicated.
"""

import numpy as np

ETA = 0.01
B, NG, HID = 32, 1024, 64
NCORES = 8
BPC = B // NCORES  # batches per core
P = 128
KT = NG // P   # 8 contraction tiles
MT = NG // P   # 8 output row tiles
NW = 512       # matmul moving free dim (one fp32 PSUM bank)
NJ = NG // NW  # 2 output col tiles

USE_F32R = True
N2 = 256        # half-tile: psum holds [re(256) | im(256)]
NJ4 = NG // N2  # 4 col tiles of 256

# Output is symmetric: keep tile (mi, nj) iff mi < 2*nj + 2 (covers the
# upper triangle); the rest is mirrored on the host.
KEEP = [(mi, nj) for mi in range(MT) for nj in range(NJ4) if mi < 2 * nj + 2]
MISS = [(mi, nj) for mi in range(MT) for nj in range(NJ4) if mi >= 2 * nj + 2]

_CACHE = {}


def _build_nc():
    from concourse import bacc
    import concourse.mybir as mybir
    import concourse.tile as tile

    f32 = mybir.dt.float32
    f32r = mybir.dt.float32r

    nc = bacc.Bacc("TRN2", target_bir_lowering=False, debug=False, num_devices=NCORES)

    qt_d = nc.dram_tensor("qt", [NG, NG], f32, kind="ExternalInput").ap()
    cre_d = nc.dram_tensor("cre", [BPC, NG], f32, kind="ExternalInput").ap()
    cim_d = nc.dram_tensor("cim", [BPC, NG], f32, kind="ExternalInput").ap()
    out_d = nc.dram_tensor("out", [BPC, NG, NG], f32, kind="ExternalOutput").ap()

    # DRAM views: k index on partitions.
    qt_v = qt_d.rearrange("(t p) m -> p t m", p=P)       # [128, KT, NG]
    cre_v = cre_d.rearrange("b (t p) -> b p t", p=P)     # [BPC, 128, KT]
    cim_v = cim_d.rearrange("b (t p) -> b p t", p=P)

    mdt = f32r if USE_F32R else f32

    with tile.TileContext(nc) as tc:
        with (
            tc.tile_pool(name="qtp", bufs=1) as qtp,
            tc.tile_pool(name="stg", bufs=2) as stg,
            tc.tile_pool(name="scp", bufs=2) as scp,
            tc.tile_pool(name="cvp", bufs=2) as cvp,
            tc.tile_pool(name="otp", bufs=3) as otp,
            tc.tile_pool(name="psp", bufs=6, space="PSUM") as psp,
        ):
            # c vectors first (tiny, gate the first scale ops)
            cts = []
            for b in range(BPC):
                cre_t = cvp.tile([P, KT], f32, tag=f"cre{b}")
                cim_t = cvp.tile([P, KT], f32, tag=f"cim{b}")
                nc.sync.dma_start(cre_t[:], cre_v[b])
                nc.sync.dma_start(cim_t[:], cim_v[b])
                cts.append((cre_t, cim_t))

            # qt: split each k-tile into 8 column chunks so chunks spread
            # across DMA queues and early k-tiles land fast
            qt = qtp.tile([P, KT, NG], mdt)
            CH = NG // 4  # 256-col chunks keep 1KB DMA packets
            for ki in range(KT):
                for c in range(4):
                    cs = slice(c * CH, (c + 1) * CH)
                    nc.sync.dma_start(qt[:, ki, cs], qt_v[:, ki, cs].bitcast(mdt))

            for b in range(BPC):
                cre_t, cim_t = cts[b]

                # scat[:, ki, nj, 0:256] = cre * qt cols, [..., 256:512] = cim * qt
                scat = scp.tile([P, KT, NJ4, 2 * N2], mdt, tag="scat")
                for ki in range(KT):
                    qv = qt[:, ki, :].rearrange("p (a b) -> p a b", b=N2)
                    nc.vector.tensor_scalar_mul(
                        scat[:, ki, :, 0:N2], qv, cre_t[:, ki : ki + 1]
                    )
                    if b == 0:
                        # startup: split scale supply across two engines
                        nc.scalar.mul(
                            scat[:, ki, :, N2 : 2 * N2], qv, cim_t[:, ki : ki + 1]
                        )
                    else:
                        nc.vector.tensor_scalar_mul(
                            scat[:, ki, :, N2 : 2 * N2], qv, cim_t[:, ki : ki + 1]
                        )

                def abs_chain(ps, mi, nj):
                    ms = slice(mi * P, (mi + 1) * P)
                    sq = otp.tile([P, 2 * N2], f32, tag="sq")
                    nc.scalar.square(sq[:], ps[:])
                    t = otp.tile([P, N2], f32, tag="t")
                    nc.vector.tensor_add(t[:], sq[:, 0:N2], sq[:, N2 : 2 * N2])
                    o = otp.tile([P, N2], f32, tag="o")
                    nc.scalar.sqrt(o[:], t[:])
                    nc.sync.dma_start(out_d[b, ms, nj * N2 : (nj + 1) * N2], o[:])

                if b == 0:
                    # first wave: 6 tiles in ki-lockstep so the PE consumes
                    # each k-level as its DMA+scale lands
                    WV = 6
                    wave = KEEP[:WV]
                    pss = []
                    for _wi in range(WV):
                        ps_w = psp.tile([P, 2 * N2], f32, tag="ps")
                        pss.append(ps_w)
                    for ki in range(KT):
                        for wi, (mi, nj) in enumerate(wave):
                            nc.tensor.matmul(
                                pss[wi][:],
                                qt[:, ki, mi * P : (mi + 1) * P],
                                scat[:, ki, nj, :],
                                start=(ki == 0),
                                stop=(ki == KT - 1),
                            )
                    for wi, (mi, nj) in enumerate(wave):
                        abs_chain(pss[wi], mi, nj)
                    rest = KEEP[WV:]
                else:
                    rest = KEEP

                for mi, nj in rest:
                    ps = psp.tile([P, 2 * N2], f32, tag="ps")
                    for ki in range(KT):
                        nc.tensor.matmul(
                            ps[:],
                            qt[:, ki, mi * P : (mi + 1) * P],
                            scat[:, ki, nj, :],
                            start=(ki == 0),
                            stop=(ki == KT - 1),
                        )
                    abs_chain(ps, mi, nj)

    nc.compile()
    return nc


def _host_prep(gene_state, H, W1, b1, W2, b2):
    # omega_net MLP -> per-batch scalar w (fp32, matching the jax reference)
    gs = gene_state.astype(np.float32).reshape(-1, HID)
    h = gs @ W1.astype(np.float32) + b1.astype(np.float32)
    h = h * (1.0 / (1.0 + np.exp(-h, dtype=np.float32)))  # SiLU
    omega = (h @ W2.astype(np.float32) + b2.astype(np.float32)).reshape(B, NG)
    w = omega.mean(axis=1)  # [B]

    Hs = 0.5 * (H.astype(np.float64) + H.astype(np.float64).T)
    lam, Q = np.linalg.eigh(Hs)  # Hs = Q diag(lam) Q^T

    d = w.astype(np.float64)[:, None] - lam[None, :]  # [B, NG]
    den = d * d + ETA * ETA
    cre = (d / den).astype(np.float32)
    cim = (-ETA / den).astype(np.float32)
    qt = np.ascontiguousarray(Q.T.astype(np.float32))  # [k, n]
    return qt, cre, cim


def kernel(gene_state, H, W1, b1, W2, b2):
    from concourse.bass_utils import run_bass_kernel_spmd

    qt, cre, cim = _host_prep(gene_state, H, W1, b1, W2, b2)

    if "nc" not in _CACHE:
        _CACHE["nc"] = _build_nc()
    nc = _CACHE["nc"]

    in_maps = [
        {
            "qt": qt,
            "cre": np.ascontiguousarray(cre[c * BPC : (c + 1) * BPC]),
            "cim": np.ascontiguousarray(cim[c * BPC : (c + 1) * BPC]),
        }
        for c in range(NCORES)
    ]
    res = run_bass_kernel_spmd(nc, in_maps, core_ids=list(range(NCORES)))
    out = np.concatenate([r["out"] for r in res.results], axis=0)
    # Mirror the skipped lower-triangle tiles from the computed upper ones.
    for mi, nj in MISS:
        r0, r1 = mi * P, (mi + 1) * P
        c0, c1 = nj * N2, (nj + 1) * N2
        out[:, r0:r1, c0:c1] = out[:, c0:c1, r0:r1].swapaxes(1, 2)
    return out


# revision 23
# speedup vs baseline: 1.1755x; 1.0302x over previous
"""Green's function layer kernel for Trainium2 (8 NeuronCores, data-parallel over batch).

Math: reference computes, per batch b,
    G_b = inv((w_b + i*eta) I - H_sym),  output |G_b|,
with H_sym = 0.5(H+H^T) shared across the batch and w_b a scalar from a tiny MLP.

Since H_sym is real symmetric and shared, eigendecompose once on host:
    H_sym = Q diag(lam) Q^T  =>  G_b = Q diag(1/(w_b - lam + i*eta)) Q^T.
With c_b = 1/(w_b - lam + i*eta) (complex vector), the per-batch work becomes two
real [1024x1024] matmuls (real and imaginary parts) plus an elementwise abs:
    Re(G_b) = Q diag(c_re) Q^T,  Im(G_b) = Q diag(c_im) Q^T,
    |G_b| = sqrt(Re^2 + Im^2).
Each core handles 4 of the 32 batches; Q^T is replicated.
"""

import numpy as np

ETA = 0.01
B, NG, HID = 32, 1024, 64
NCORES = 8
BPC = B // NCORES  # batches per core
P = 128
KT = NG // P   # 8 contraction tiles
MT = NG // P   # 8 output row tiles
NW = 512       # matmul moving free dim (one fp32 PSUM bank)
NJ = NG // NW  # 2 output col tiles

USE_F32R = True
N2 = 256        # half-tile: psum holds [re(256) | im(256)]
NJ4 = NG // N2  # 4 col tiles of 256

# Output is symmetric: keep tile (mi, nj) iff mi < 2*nj + 2 (covers the
# upper triangle); the rest is mirrored on the host.
KEEP = [(mi, nj) for mi in range(MT) for nj in range(NJ4) if mi < 2 * nj + 2]
MISS = [(mi, nj) for mi in range(MT) for nj in range(NJ4) if mi >= 2 * nj + 2]

_CACHE = {}


def _build_nc():
    from concourse import bacc
    import concourse.mybir as mybir
    import concourse.tile as tile

    f32 = mybir.dt.float32
    f32r = mybir.dt.float32r

    nc = bacc.Bacc("TRN2", target_bir_lowering=False, debug=False, num_devices=NCORES)

    qt_d = nc.dram_tensor("qt", [NG, NG], f32, kind="ExternalInput").ap()
    # cc rows: [cre(b=0..3), cim(b=0..3)], each [NG]
    cc_d = nc.dram_tensor("cc", [2 * BPC, NG], f32, kind="ExternalInput").ap()
    out_d = nc.dram_tensor("out", [BPC, NG, NG], f32, kind="ExternalOutput").ap()

    # DRAM view: k index on partitions.
    qt_v = qt_d.rearrange("(t p) m -> p t m", p=P)       # [128, KT, NG]

    mdt = f32r if USE_F32R else f32

    with tile.TileContext(nc) as tc:
        with (
            tc.tile_pool(name="qtp", bufs=1) as qtp,
            tc.tile_pool(name="stg", bufs=2) as stg,
            tc.tile_pool(name="scp", bufs=2) as scp,
            tc.tile_pool(name="cvp", bufs=2) as cvp,
            tc.tile_pool(name="otp", bufs=3) as otp,
            tc.tile_pool(name="psp", bufs=6, space="PSUM") as psp,
            tc.tile_pool(name="psc", bufs=1, space="PSUM") as psc,
        ):
            # c vectors: one contiguous DMA, then PE-transpose into
            # per-partition layout cvec[p, t, v] = cc[v, t*128+p]
            NV = 2 * BPC
            cc_sb = cvp.tile([NV, NG], f32, tag="cc")
            nc.sync.dma_start(cc_sb[:], cc_d)
            id8 = cvp.tile([NV, NV], f32, tag="id8")
            from concourse.masks import make_identity

            make_identity(nc, id8[:])
            ct_ps = psc.tile([P, KT, NV], f32, tag="ct")
            for t in range(KT):
                nc.tensor.transpose(
                    ct_ps[:, t, :], cc_sb[:, t * P : (t + 1) * P], id8[:]
                )
            cvec = cvp.tile([P, KT, NV], f32, tag="cvec")
            nc.vector.tensor_copy(cvec[:], ct_ps[:])

            # qt: split each k-tile into 8 column chunks so chunks spread
            # across DMA queues and early k-tiles land fast
            qt = qtp.tile([P, KT, NG], mdt)
            CH = NG // 4  # 256-col chunks keep 1KB DMA packets
            for ki in range(KT):
                for c in range(4):
                    cs = slice(c * CH, (c + 1) * CH)
                    nc.sync.dma_start(qt[:, ki, cs], qt_v[:, ki, cs].bitcast(mdt))

            for b in range(BPC):
                # scat[:, ki, nj, 0:256] = cre * qt cols, [..., 256:512] = cim * qt
                scat = scp.tile([P, KT, NJ4, 2 * N2], mdt, tag="scat")
                for ki in range(KT):
                    qv = qt[:, ki, :].rearrange("p (a b) -> p a b", b=N2)
                    cre_s = cvec[:, ki, b : b + 1]
                    cim_s = cvec[:, ki, BPC + b : BPC + b + 1]
                    nc.vector.tensor_scalar_mul(scat[:, ki, :, 0:N2], qv, cre_s)
                    if b == 0:
                        # startup: split scale supply across two engines
                        nc.scalar.mul(scat[:, ki, :, N2 : 2 * N2], qv, cim_s)
                    else:
                        nc.vector.tensor_scalar_mul(
                            scat[:, ki, :, N2 : 2 * N2], qv, cim_s
                        )

                def abs_chain(ps, mi, nj):
                    ms = slice(mi * P, (mi + 1) * P)
                    sq = otp.tile([P, 2 * N2], f32, tag="sq")
                    nc.scalar.square(sq[:], ps[:])
                    t = otp.tile([P, N2], f32, tag="t")
                    nc.vector.tensor_add(t[:], sq[:, 0:N2], sq[:, N2 : 2 * N2])
                    o = otp.tile([P, N2], f32, tag="o")
                    nc.scalar.sqrt(o[:], t[:])
                    nc.sync.dma_start(out_d[b, ms, nj * N2 : (nj + 1) * N2], o[:])

                if b == 0:
                    # first wave: 6 tiles in ki-lockstep so the PE consumes
                    # each k-level as its DMA+scale lands
                    WV = 6
                    wave = KEEP[:WV]
                    pss = []
                    for _wi in range(WV):
                        ps_w = psp.tile([P, 2 * N2], f32, tag="ps")
                        pss.append(ps_w)
                    for ki in range(KT):
                        for wi, (mi, nj) in enumerate(wave):
                            nc.tensor.matmul(
                                pss[wi][:],
                                qt[:, ki, mi * P : (mi + 1) * P],
                                scat[:, ki, nj, :],
                                start=(ki == 0),
                                stop=(ki == KT - 1),
                            )
                    for wi, (mi, nj) in enumerate(wave):
                        abs_chain(pss[wi], mi, nj)
                    rest = KEEP[WV:]
                else:
                    rest = KEEP

                for mi, nj in rest:
                    ps = psp.tile([P, 2 * N2], f32, tag="ps")
                    for ki in range(KT):
                        nc.tensor.matmul(
                            ps[:],
                            qt[:, ki, mi * P : (mi + 1) * P],
                            scat[:, ki, nj, :],
                            start=(ki == 0),
                            stop=(ki == KT - 1),
                        )
                    abs_chain(ps, mi, nj)

    nc.compile()
    return nc


def _host_prep(gene_state, H, W1, b1, W2, b2):
    # omega_net MLP -> per-batch scalar w (fp32, matching the jax reference)
    gs = gene_state.astype(np.float32).reshape(-1, HID)
    h = gs @ W1.astype(np.float32) + b1.astype(np.float32)
    h = h * (1.0 / (1.0 + np.exp(-h, dtype=np.float32)))  # SiLU
    omega = (h @ W2.astype(np.float32) + b2.astype(np.float32)).reshape(B, NG)
    w = omega.mean(axis=1)  # [B]

    Hs = 0.5 * (H.astype(np.float64) + H.astype(np.float64).T)
    lam, Q = np.linalg.eigh(Hs)  # Hs = Q diag(lam) Q^T

    d = w.astype(np.float64)[:, None] - lam[None, :]  # [B, NG]
    den = d * d + ETA * ETA
    cre = (d / den).astype(np.float32)
    cim = (-ETA / den).astype(np.float32)
    qt = np.ascontiguousarray(Q.T.astype(np.float32))  # [k, n]
    return qt, cre, cim


def kernel(gene_state, H, W1, b1, W2, b2):
    from concourse.bass_utils import run_bass_kernel_spmd

    qt, cre, cim = _host_prep(gene_state, H, W1, b1, W2, b2)

    if "nc" not in _CACHE:
        _CACHE["nc"] = _build_nc()
    nc = _CACHE["nc"]

    in_maps = [
        {
            "qt": qt,
            "cc": np.ascontiguousarray(
                np.concatenate(
                    [cre[c * BPC : (c + 1) * BPC], cim[c * BPC : (c + 1) * BPC]], axis=0
                )
            ),
        }
        for c in range(NCORES)
    ]
    res = run_bass_kernel_spmd(nc, in_maps, core_ids=list(range(NCORES)))
    out = np.concatenate([r["out"] for r in res.results], axis=0)
    # Mirror the skipped lower-triangle tiles from the computed upper ones.
    for mi, nj in MISS:
        r0, r1 = mi * P, (mi + 1) * P
        c0, c1 = nj * N2, (nj + 1) * N2
        out[:, r0:r1, c0:c1] = out[:, c0:c1, r0:r1].swapaxes(1, 2)
    return out
